# revision 28
# baseline (speedup 1.0000x reference)
"""Bass/Trainium2 kernel for nn_EquivariantReynoldsWrap.

The reference module is linear in x: for every pixel,
    out = (1/G) * sum_g BlockDiag(A_g) @ Wf @ BlockDiag(Ainv_g) @ x_pixel
so the whole pipeline collapses into one 64x64 channel-mixing matrix M,
computed on host (cheap). The device work is a single 1x1-conv matmul
out[b] = M @ x[b] with x[b] viewed as (64, H*W).

Sharding: data-parallel over B across the 8 cores (1 batch each).
Per core the two halves of the pixel axis are interleaved on the
partition axis (partition p = channel p//2, half p%2) and the stationary
weight is the 128x128 interleaved block-diagonal of M^T, so each
512-column matmul covers 1024 pixels.

I/O in bf16 (half the DMA bytes of f32; the 2e-2 accuracy budget is
~10x above bf16's ~2e-3; PE runs single-pass instead of fp32's
LOW/HIGH double pass). Measured structure on HW:
  - ~6.9us fixed NEFF preamble before the first DMA trigger, data
    lands from ~8.4us; input streams at ~230-240 GB/s aggregate.
  - the weight tile rides the pool (SWDGE) ring alone-first: its
    completion sem on a shared HW ring only lands after ALL later
    transfers on that ring. Pool also carries the last x chunk,
    freeing the two HWDGE rings (sync: x0+x2, scalar: x1).
  - chunk matmuls (427ns/512col; PE column clock 1.2GHz) gate on the
    per-chunk DMA sems (~0.7-0.9us completion->sem propagation).
  - copies gate on each matmul's own retire-inc: the copy engines'
    slower column rate (1.35ns/col vs the drain's 0.83) never catches
    the ~128-column systolic drain, so no guard matmul is needed.
  - copies alternate DVE (chunks 0,2) / ACT (1,3); out-triggers:
    sync {y0, y1, y2}, scalar {y3 right after its copy}. One PSUM
    bank (512 f32 cols) per chunk, never touched by two engines
    concurrently (same-bank sharing wedges the device).
  - 4 warm-up matmuls on garbage ramp the PE clock (cold PE runs
    ~1.5ns/col); their results go to a never-read PSUM tile.

Raw bacc (no TileContext): hand-rolled semaphores, minimal head/tail.
"""

import numpy as np
import ml_dtypes

import concourse.bacc as bacc
import concourse.bass as bass
from concourse import mybir
from concourse.bass_utils import run_bass_kernel_spmd

B, C, H, W_SP = 8, 64, 64, 64
COUT = 64
HW = H * W_SP          # 4096 pixels per batch
HALF = HW // 2         # 2048 -> stacked column count per core
N_CORES = 8

CH = 512               # columns per pipeline chunk
N_CHUNKS = HALF // CH  # 4
HC = CH // 2           # copy split point within a chunk
N_WARM = 6             # bf16 warm-up matmuls (HAM un-throttle)

TRACE = False          # test.py flips this to profile
_cached_nc = None

BF16 = ml_dtypes.bfloat16


def _build_nc():
    global _cached_nc
    if _cached_nc is not None:
        return _cached_nc

    bf16 = mybir.dt.bfloat16
    f32 = mybir.dt.float32

    nc = bacc.Bacc(
        "TRN2",
        target_bir_lowering=False,
        debug=False,
        enable_asserts=False,
        num_devices=N_CORES,
    )
    xd = nc.dram_tensor("x", [C, HW], bf16, kind="ExternalInput").ap()
    wd = nc.dram_tensor("w", [128, 128], bf16, kind="ExternalInput").ap()
    yd = nc.dram_tensor("y", [COUT, HW], bf16, kind="ExternalOutput").ap()

    # [64, 2, t] c-major outer dims: the DMA pairs partition p with
    # (c=p//2, s=p%2); the outer dim of 64 spreads each transfer across
    # all 16 SDMA engines (an outer dim of 2 used only 2 of them).
    xr = xd.rearrange("c (s t) -> c s t", s=2)
    yr = yd.rearrange("c (s t) -> c s t", s=2)

    with (
        nc.sbuf_tensor("wt", [128, 128], bf16) as wt_t,
        nc.sbuf_tensor("xt", [128, HALF], bf16) as xt_t,
        nc.sbuf_tensor("ot", [128, HALF], bf16) as ot_t,
        nc.sbuf_tensor("zt", [128, 512], mybir.dt.bfloat16) as zt_t,
        nc.psum_tensor([128, HALF], f32) as ps_t,
        nc.psum_tensor([128, 512], f32) as wps_t,
        nc.semaphore("s_w") as s_w,      # weights DMA done
        # one sem per x-chunk DMA: a sem shared by two DMAs on one ring
        # reaches 16 from a MIX of the two transfers' per-engine incs
        nc.semaphore("s_x0") as s_x0,
        nc.semaphore("s_x1") as s_x1,
        nc.semaphore("s_x2") as s_x2,
        nc.semaphore("s_x3") as s_x3,
        nc.semaphore("s_z") as s_z,      # warmup tile zeroed
        nc.semaphore("s_mm") as s_mm,    # matmul+guard pairs (2 per chunk)
        nc.semaphore("s_c0") as s_c0,    # chunk copy done (2 halves)
        nc.semaphore("s_c1") as s_c1,
        nc.semaphore("s_c2") as s_c2,
        nc.semaphore("s_c3") as s_c3,
        nc.semaphore("s_y") as s_y,      # out DMAs
    ):
        wt = wt_t.ap()
        xt = xt_t.ap()
        ot = ot_t.ap()
        zt = zt_t.ap()
        ps = ps_t.ap()
        wps = wps_t.ap()

        def cs(i):
            return slice(i * CH, (i + 1) * CH)

        def csl(i):  # low copy half
            return slice(i * CH, i * CH + HC)

        def csh(i):  # high copy half
            return slice(i * CH + HC, (i + 1) * CH)

        # Linear emission into the entry basic block (no nc.Block): avoids
        # the per-engine body branches (I$ misses) and the Block exit
        # barrier; the walrus-generated NEFF epilogue handles quiescence
        # and zeroes all semaphores for re-execution.
        sync, scalar, tensor, vector, gpsimd = (
            nc.sync, nc.scalar, nc.tensor, nc.vector, nc.gpsimd
        )

        # ring assignment: pool takes w (alone-first, for an early w-sem)
        # then x3; sync takes x0 + x2; scalar takes x1. Chunk index ==
        # expected arrival order, which the matmul queue follows.
        gpsimd.dma_start(wt[:], wd[:]).then_inc(s_w, 16)
        gpsimd.dma_start(xt[:, cs(3)], xr[:, :, cs(3)]).then_inc(s_x3, 16)
        sync.dma_start(xt[:, cs(0)], xr[:, :, cs(0)]).then_inc(s_x0, 16)
        sync.dma_start(xt[:, cs(2)], xr[:, :, cs(2)]).then_inc(s_x2, 16)
        scalar.dma_start(xt[:, cs(1)], xr[:, :, cs(1)]).then_inc(s_x1, 16)

        # warm-up matmuls on the (uninitialized) zt tile ramp the PE
        # clock; results go to wps which is never read, so garbage
        # inputs are fine.
        for _ in range(N_WARM):
            tensor.matmul(wps[:], zt[:, :128], zt[:])

        # copies gate on each matmul's own retire-inc; a matmul's sem
        # fires when the last column ENTERS the array, but the copy
        # engines' slower column rate never catches the ~128-column
        # systolic drain, so no guard matmul is needed.
        tensor.wait_ge(s_w, 16)
        xs = [s_x0, s_x1, s_x2, s_x3]
        for i in range(N_CHUNKS):
            tensor.wait_ge(xs[i], 16)
            tensor.matmul(ps[:, cs(i)], wt[:], xt[:, cs(i)]).then_inc(s_mm)

        # copies (cast f32 PSUM -> bf16 SBUF): DVE takes chunks 0, 2; ACT
        # takes 1, 3.
        vector.wait_ge(s_mm, 1)
        vector.tensor_copy(ot[:, cs(0)], ps[:, cs(0)]).then_inc(s_c0)
        vector.wait_ge(s_mm, 3)
        vector.tensor_copy(ot[:, cs(2)], ps[:, cs(2)]).then_inc(s_c2)

        scalar.wait_ge(s_mm, 2)
        scalar.copy(ot[:, cs(1)], ps[:, cs(1)]).then_inc(s_c1)
        scalar.wait_ge(s_mm, 4)
        scalar.copy(ot[:, cs(3)], ps[:, cs(3)]).then_inc(s_c3)
        scalar.wait_ge(s_c3, 1)
        scalar.dma_start(yr[:, :, cs(3)], ot[:, cs(3)]).then_inc(s_y, 16)

        sync.wait_ge(s_c0, 1)
        sync.dma_start(yr[:, :, cs(0)], ot[:, cs(0)]).then_inc(s_y, 16)
        sync.wait_ge(s_c1, 1)
        sync.dma_start(yr[:, :, cs(1)], ot[:, cs(1)]).then_inc(s_y, 16)
        sync.wait_ge(s_c2, 1)
        sync.dma_start(yr[:, :, cs(2)], ot[:, cs(2)]).then_inc(s_y, 16)
        # the NEFF epilogue's per-ring DGE drains hold teardown until all
        # output descriptors (data + sem incs) have retired
        _ = s_y

    nc.compile()
    _cached_nc = nc
    return nc


def _fuse_weights(group_tensor, group_tensor_inv, Wf):
    A = np.asarray(group_tensor, np.float64)
    Ai = np.asarray(group_tensor_inv, np.float64)
    Wf64 = np.asarray(Wf, np.float64)
    G, CG, _ = A.shape
    n = C // CG
    eye = np.eye(n)
    M = np.zeros((COUT, C))
    for g in range(G):
        M += np.kron(eye, A[g]) @ Wf64 @ np.kron(eye, Ai[g])
    M /= G
    MT = np.ascontiguousarray(M.T).astype(np.float32)
    # interleaved packing: x-tile partition p holds channel p//2 of pixel
    # half p%2; out partition q holds channel q//2 of half q%2.
    W2T = np.zeros((128, 128), np.float32)
    W2T[0::2, 0::2] = MT
    W2T[1::2, 1::2] = MT
    return W2T.astype(BF16)


def kernel(x, group_tensor, group_tensor_inv, Wf):
    nc = _build_nc()
    W2T = _fuse_weights(group_tensor, group_tensor_inv, Wf)
    x = np.ascontiguousarray(np.asarray(x, np.float32).astype(BF16))

    in_maps = [
        {"x": x[b].reshape(C, HW), "w": W2T} for b in range(B)
    ]
    res = run_bass_kernel_spmd(
        nc, in_maps, core_ids=list(range(N_CORES)), trace=TRACE
    )
    if TRACE:
        kernel.last_results = res
    y = np.stack(
        [
            res.results[b]["y"].astype(np.float32).reshape(COUT, H, W_SP)
            for b in range(B)
        ]
    )
    return y


# revision 31
# speedup vs baseline: 1.0205x; 1.0205x over previous
"""Bass/Trainium2 kernel for nn_EquivariantReynoldsWrap.

The reference module is linear in x: for every pixel,
    out = (1/G) * sum_g BlockDiag(A_g) @ Wf @ BlockDiag(Ainv_g) @ x_pixel
so the whole pipeline collapses into one 64x64 channel-mixing matrix M,
computed on host (cheap). The device work is a single 1x1-conv matmul
out[b] = M @ x[b] with x[b] viewed as (64, H*W).

Sharding: data-parallel over B across the 8 cores (1 batch each).
Per core the two halves of the pixel axis are interleaved on the
partition axis (partition p = channel p//2, half p%2) and the stationary
weight is the 128x128 interleaved block-diagonal of M^T, so each
512-column matmul covers 1024 pixels.

I/O in bf16 (half the DMA bytes of f32; the 2e-2 accuracy budget is
~10x above bf16's ~2e-3; PE runs single-pass instead of fp32's
LOW/HIGH double pass). Measured structure on HW:
  - ~6.9us fixed NEFF preamble before the first DMA trigger, data
    lands from ~8.4us; input streams at ~230-240 GB/s aggregate.
  - the weight tile rides the pool (SWDGE) ring alone-first: its
    completion sem on a shared HW ring only lands after ALL later
    transfers on that ring. Pool also carries the last x chunk,
    freeing the two HWDGE rings (sync: x0+x2, scalar: x1).
  - chunk matmuls (427ns/512col; PE column clock 1.2GHz) gate on the
    per-chunk DMA sems (~0.7-0.9us completion->sem propagation).
  - copies gate on each matmul's own retire-inc: the copy engines'
    slower column rate (1.35ns/col vs the drain's 0.83) never catches
    the ~128-column systolic drain, so no guard matmul is needed.
  - copies alternate DVE (chunks 0,2) / ACT (1,3); out-triggers:
    sync {y0, y1, y2}, scalar {y3 right after its copy}. One PSUM
    bank (512 f32 cols) per chunk, never touched by two engines
    concurrently (same-bank sharing wedges the device).
  - 6 warm-up matmuls on garbage ramp the PE clock (cold PE runs
    ~1.5ns/col); their results go to a never-read PSUM tile. They end
    ~8.9us, well before the w-sem gate (~10.2us), so they are free.

Raw bacc (no TileContext): hand-rolled semaphores, minimal head/tail.
"""

import numpy as np
import ml_dtypes

import concourse.bacc as bacc
import concourse.bass as bass
from concourse import mybir
from concourse.bass_utils import run_bass_kernel_spmd

B, C, H, W_SP = 8, 64, 64, 64
COUT = 64
HW = H * W_SP          # 4096 pixels per batch
HALF = HW // 2         # 2048 -> stacked column count per core
N_CORES = 8

CH = 512               # columns per pipeline chunk
N_CHUNKS = HALF // CH  # 4
HC = CH // 2           # copy split point within a chunk
N_WARM = 6             # bf16 warm-up matmuls (HAM un-throttle)

TRACE = False          # test.py flips this to profile
_cached_nc = None

BF16 = ml_dtypes.bfloat16


def _build_nc():
    global _cached_nc
    if _cached_nc is not None:
        return _cached_nc

    bf16 = mybir.dt.bfloat16
    f32 = mybir.dt.float32

    nc = bacc.Bacc(
        "TRN2",
        target_bir_lowering=False,
        debug=False,
        enable_asserts=False,
        num_devices=N_CORES,
    )
    xd = nc.dram_tensor("x", [C, HW], bf16, kind="ExternalInput").ap()
    wd = nc.dram_tensor("w", [128, 128], bf16, kind="ExternalInput").ap()
    yd = nc.dram_tensor("y", [COUT, HW], bf16, kind="ExternalOutput").ap()

    # [64, 2, t] c-major outer dims: the DMA pairs partition p with
    # (c=p//2, s=p%2); the outer dim of 64 spreads each transfer across
    # all 16 SDMA engines (an outer dim of 2 used only 2 of them).
    xr = xd.rearrange("c (s t) -> c s t", s=2)
    yr = yd.rearrange("c (s t) -> c s t", s=2)

    with (
        nc.sbuf_tensor("wt", [128, 128], bf16) as wt_t,
        nc.sbuf_tensor("xt", [128, HALF], bf16) as xt_t,
        nc.sbuf_tensor("ot", [128, HALF], bf16) as ot_t,
        nc.sbuf_tensor("zt", [128, 512], mybir.dt.bfloat16) as zt_t,
        nc.psum_tensor([128, HALF + 512], f32) as ps_t,
        nc.psum_tensor([128, 512], f32) as wps_t,
        nc.semaphore("s_w") as s_w,      # weights DMA done
        # one sem per x-chunk DMA: a sem shared by two DMAs on one ring
        # reaches 16 from a MIX of the two transfers' per-engine incs
        nc.semaphore("s_x0") as s_x0,
        nc.semaphore("s_x1") as s_x1,
        nc.semaphore("s_x2") as s_x2,
        nc.semaphore("s_x3") as s_x3,
        nc.semaphore("s_x3b") as s_x3b,
        nc.semaphore("s_z") as s_z,      # warmup tile zeroed
        nc.semaphore("s_mm") as s_mm,    # matmul+guard pairs (2 per chunk)
        nc.semaphore("s_c0") as s_c0,    # chunk copy done (2 halves)
        nc.semaphore("s_c1") as s_c1,
        nc.semaphore("s_c2") as s_c2,
        nc.semaphore("s_c3") as s_c3,
        nc.semaphore("s_y") as s_y,      # out DMAs
    ):
        wt = wt_t.ap()
        xt = xt_t.ap()
        ot = ot_t.ap()
        zt = zt_t.ap()
        ps = ps_t.ap()
        wps = wps_t.ap()

        def cs(i):
            return slice(i * CH, (i + 1) * CH)

        def csl(i):  # low copy half
            return slice(i * CH, i * CH + HC)

        def csh(i):  # high copy half
            return slice(i * CH + HC, (i + 1) * CH)

        # Linear emission into the entry basic block (no nc.Block): avoids
        # the per-engine body branches (I$ misses) and the Block exit
        # barrier; the walrus-generated NEFF epilogue handles quiescence
        # and zeroes all semaphores for re-execution.
        sync, scalar, tensor, vector, gpsimd = (
            nc.sync, nc.scalar, nc.tensor, nc.vector, nc.gpsimd
        )

        # ring assignment: pool takes w (alone-first, for an early w-sem)
        # then x3; sync takes x0 + x2; scalar takes x1. Chunk index ==
        # expected arrival order, which the matmul queue follows.
        c3a = slice(3 * CH, 3 * CH + CH // 2)
        c3b = slice(3 * CH + CH // 2, 4 * CH)
        gpsimd.dma_start(wt[:], wd[:]).then_inc(s_w, 16)
        gpsimd.dma_start(xt[:, c3b], xr[:, :, c3b]).then_inc(s_x3b, 16)
        sync.dma_start(xt[:, cs(0)], xr[:, :, cs(0)]).then_inc(s_x0, 16)
        sync.dma_start(xt[:, cs(2)], xr[:, :, cs(2)]).then_inc(s_x2, 16)
        scalar.dma_start(xt[:, cs(1)], xr[:, :, cs(1)]).then_inc(s_x1, 16)
        scalar.dma_start(xt[:, c3a], xr[:, :, c3a]).then_inc(s_x3, 16)

        # warm-up matmuls on the (uninitialized) zt tile ramp the PE
        # clock; results go to wps which is never read, so garbage
        # inputs are fine.
        for _ in range(N_WARM):
            tensor.matmul(wps[:], zt[:, :128], zt[:])

        # copies gate on each matmul's own retire-inc; a matmul's sem
        # fires when the last column ENTERS the array, but the copy
        # engines' slower column rate never catches the ~128-column
        # systolic drain, so no guard matmul is needed.
        tensor.wait_ge(s_w, 16)
        for i in range(3):
            tensor.wait_ge([s_x0, s_x1, s_x2][i], 16)
            tensor.matmul(ps[:, cs(i)], wt[:], xt[:, cs(i)]).then_inc(s_mm)
        tensor.wait_ge(s_x3, 16)
        tensor.matmul(ps[:, c3a], wt[:], xt[:, c3a]).then_inc(s_mm)
        p3b = slice(4 * CH, 4 * CH + CH // 2)
        tensor.wait_ge(s_x3b, 16)
        tensor.matmul(ps[:, p3b], wt[:], xt[:, c3b]).then_inc(s_mm)

        # copies (cast f32 PSUM -> bf16 SBUF): DVE takes chunks 0, 2; ACT
        # takes 1, 3.
        vector.wait_ge(s_mm, 1)
        vector.tensor_copy(ot[:, cs(0)], ps[:, cs(0)]).then_inc(s_c0)
        vector.wait_ge(s_mm, 3)
        vector.tensor_copy(ot[:, cs(2)], ps[:, cs(2)]).then_inc(s_c2)

        scalar.wait_ge(s_mm, 2)
        scalar.copy(ot[:, cs(1)], ps[:, cs(1)]).then_inc(s_c1)
        scalar.wait_ge(s_mm, 4)
        scalar.copy(ot[:, c3a], ps[:, c3a]).then_inc(s_c3)
        scalar.wait_ge(s_mm, 5)
        scalar.copy(ot[:, c3b], ps[:, p3b]).then_inc(s_c3)
        scalar.wait_ge(s_c3, 1)
        scalar.dma_start(yr[:, :, c3a], ot[:, c3a]).then_inc(s_y, 16)
        scalar.wait_ge(s_c3, 2)
        scalar.dma_start(yr[:, :, c3b], ot[:, c3b]).then_inc(s_y, 16)

        sync.wait_ge(s_c0, 1)
        sync.dma_start(yr[:, :, cs(0)], ot[:, cs(0)]).then_inc(s_y, 16)
        sync.wait_ge(s_c1, 1)
        sync.dma_start(yr[:, :, cs(1)], ot[:, cs(1)]).then_inc(s_y, 16)
        sync.wait_ge(s_c2, 1)
        sync.dma_start(yr[:, :, cs(2)], ot[:, cs(2)]).then_inc(s_y, 16)
        # the NEFF epilogue's per-ring DGE drains hold teardown until all
        # output descriptors (data + sem incs) have retired
        _ = s_y

    nc.compile()
    _cached_nc = nc
    return nc


def _fuse_weights(group_tensor, group_tensor_inv, Wf):
    A = np.asarray(group_tensor, np.float64)
    Ai = np.asarray(group_tensor_inv, np.float64)
    Wf64 = np.asarray(Wf, np.float64)
    G, CG, _ = A.shape
    n = C // CG
    eye = np.eye(n)
    M = np.zeros((COUT, C))
    for g in range(G):
        M += np.kron(eye, A[g]) @ Wf64 @ np.kron(eye, Ai[g])
    M /= G
    MT = np.ascontiguousarray(M.T).astype(np.float32)
    # interleaved packing: x-tile partition p holds channel p//2 of pixel
    # half p%2; out partition q holds channel q//2 of half q%2.
    W2T = np.zeros((128, 128), np.float32)
    W2T[0::2, 0::2] = MT
    W2T[1::2, 1::2] = MT
    return W2T.astype(BF16)


def kernel(x, group_tensor, group_tensor_inv, Wf):
    nc = _build_nc()
    W2T = _fuse_weights(group_tensor, group_tensor_inv, Wf)
    x = np.ascontiguousarray(np.asarray(x, np.float32).astype(BF16))

    in_maps = [
        {"x": x[b].reshape(C, HW), "w": W2T} for b in range(B)
    ]
    res = run_bass_kernel_spmd(
        nc, in_maps, core_ids=list(range(N_CORES)), trace=TRACE
    )
    if TRACE:
        kernel.last_results = res
    y = np.stack(
        [
            res.results[b]["y"].astype(np.float32).reshape(COUT, H, W_SP)
            for b in range(B)
        ]
    )
    return y
